# revision 12
# baseline (speedup 1.0000x reference)
"""Trainium2 Bass kernel for nn_CrossAttention_31791347925417.

Math (per batch b, per stream tok in {x, blood} with weight W in {W1, W2}):
    kv = tok @ W.T ; k, v heads [H, N, D]
    ctx = softmax_d( SCALE * k_h^T v_h )          # [H, D, D], softmax over first D
    out_x = x_h @ ctx2_h ; out_b = blood_h @ ctx1_h

Refactor (Gram trick):
    k_h^T v_h = W_k_h (tok^T tok) W_v_h^T  with G = tok^T tok  [C, C]
so the N=4096 contraction happens once (G), and everything downstream is tiny
[C,C]-scale work.  ctxT blocks come from Q = G @ WkT then per head-pair
WvT_pair^T @ Q_pair whose diagonal 64x64 blocks are ctxT_h.  Softmax along the
free axis; normalized probs land in the diagonal blocks of a zeroed [128,128]
tile F; BD = F^T (PE transpose) is the block-diagonal ctx pair used by
    out[n, (h,e)] = sum_{(h,d)} tokT[(h,d), n] * BD[(h,d), (h,e)]

All on-chip data is fp16 (PSUM accumulation stays f32): fp16 matmuls and
transposes run at 1 cycle/column on the PE, and fp16 HBM I/O halves DMA
traffic vs f32.  Host casts inputs to fp16 and upcasts the fp16 outputs;
SCALE (2^-3, exact) folds into the k-weights.

Schedule (v3): ONE joint accumulation phase computes G_x and G_b together
(6 PSUM banks: per stream m0, m1, m2+m3 packed) plus the x transposes on PE;
blood transposes go through the DMA crossbar (dma_start_transpose) straight
from DRAM into per-group SBUF tiles, off the PE's critical path.  Chain A
runs, then out_b production is interleaved with chain B's matmuls so the ob
writes and chain B share the window; finally out_x streams out.  Both 4 MB
output streams thus pipeline against live compute instead of draining at the
end.

Sharding: data-parallel over batch B=8 across the 8 cores; weights replicated.
"""

import sys

if "/opt/trn_rl_repo" not in sys.path:
    sys.path.insert(0, "/opt/trn_rl_repo")

import numpy as np

from concourse import bacc, masks, mybir, tile
from concourse.bass_utils import run_bass_kernel_spmd

B, N, C, H = 8, 4096, 512, 8
D = C // H
SCALE = D ** -0.5
P = 128
NBIG = N // 512          # 8 big row tiles (512 rows each)
NT = N // P              # 32 n-tiles
CB = C // P              # 4 column blocks == head pairs
F32 = mybir.dt.float32
F16 = mybir.dt.float16
AX = mybir.AxisListType
ACT_EXP = mybir.ActivationFunctionType.Exp

# G is symmetric: row-block m only needs columns >= G_OFF[m]
G_OFF = [0, P, 2 * P, 3 * P]

# blood transposes via DMA crossbar (kb groups 0..XBAR_KB-1); the last group
# runs on the PE to fill the chain-A softmax bubble
XBAR_KB = NBIG - 1


def build_nc():
    nc = bacc.Bacc("TRN2", target_bir_lowering=False, debug=False)

    xb = nc.dram_tensor("xb", [N, C], F16, kind="ExternalInput").ap()
    bb = nc.dram_tensor("bb", [N, C], F16, kind="ExternalInput").ap()
    w1t = nc.dram_tensor("w1t", [C, 2 * C], F16, kind="ExternalInput").ap()
    w2t = nc.dram_tensor("w2t", [C, 2 * C], F16, kind="ExternalInput").ap()
    # blocked transposed output layout: [kb, part(c within pair), pair, n-col]
    ox = nc.dram_tensor("oxT", [NBIG, P, CB, 512], F16, kind="ExternalOutput").ap()
    ob = nc.dram_tensor("obT", [NBIG, P, CB, 512], F16, kind="ExternalOutput").ap()

    with tile.TileContext(nc) as tc:
        _emit(nc, tc, xb, bb, w1t, w2t, ox, ob)

    nc.compile()
    return nc


def _emit(nc, tc, xb, bb, w1t, w2t, ox, ob):
    from contextlib import ExitStack

    ctx = ExitStack()
    with ctx:
        const = ctx.enter_context(tc.tile_pool(name="const", bufs=1))
        wpool = ctx.enter_context(tc.tile_pool(name="wpool", bufs=1))
        tokp = ctx.enter_context(tc.tile_pool(name="tokp", bufs=16))
        xtp = ctx.enter_context(tc.tile_pool(name="xtp", bufs=1))
        xtbp = ctx.enter_context(tc.tile_pool(name="xtbp", bufs=NBIG))
        gqp = ctx.enter_context(tc.tile_pool(name="gqp", bufs=8))
        smallp = ctx.enter_context(tc.tile_pool(name="smallp", bufs=2))
        g3p = ctx.enter_context(tc.tile_pool(name="g3p", bufs=2))
        fpool = ctx.enter_context(tc.tile_pool(name="fpool", bufs=8))
        bdpool = ctx.enter_context(tc.tile_pool(name="bdpool", bufs=8))
        ostp = ctx.enter_context(tc.tile_pool(name="ostp", bufs=3))
        psG = ctx.enter_context(tc.tile_pool(name="psG", bufs=6, space="PSUM"))
        psO = ctx.enter_context(tc.tile_pool(name="psO", bufs=2, space="PSUM"))

        ident = const.tile([P, P], F16, tag="idh")
        masks.make_identity(nc, ident[:])

        # weights: chunk j (c-rows 128j..128j+128) lives at cols [j*2C, (j+1)*2C)
        w_x = wpool.tile([P, CB * 2 * C], F16, tag="wx")
        w_b = wpool.tile([P, CB * 2 * C], F16, tag="wb")

        def load_weights():
            nc.sync.dma_start(
                w_x[:].rearrange("p (j c) -> p j c", j=CB),
                w1t[:, :].rearrange("(j p) c -> p j c", p=P),
            )
            nc.sync.dma_start(
                w_b[:].rearrange("p (j c) -> p j c", j=CB),
                w2t[:, :].rearrange("(j p) c -> p j c", p=P),
            )

        def wchunk(w, j):
            return w[:, j * 2 * C:(j + 1) * 2 * C]

        # x transposed: one big tile, pair block m at cols [m*N, (m+1)*N)
        xT_x = xtp.tile([P, CB * N], F16, tag="xtx")
        # blood transposed: per-kb contiguous tiles (xbar DMA needs a
        # contiguous destination), pair block m at cols [m*512, (m+1)*512)
        xT_b = [xtbp.tile([P, CB * 512], F16, tag="xtb", name=f"xtb{kb}")
                for kb in range(NBIG)]

        def load_tok(toks, tok_dram, kb, split=False):
            tokb = tokp.tile([P, 4 * C], F16, tag="tok", name=f"tok{kb}")
            if split:
                for sub in range(4):
                    nc.sync.dma_start(
                        tokb[:, sub * C:(sub + 1) * C],
                        tok_dram[kb * 512 + sub * P:kb * 512 + (sub + 1) * P, :],
                    )
            else:
                nc.sync.dma_start(
                    tokb[:].rearrange("p (s c) -> p s c", s=4),
                    tok_dram[kb * 512:(kb + 1) * 512, :].rearrange(
                        "(s p) c -> p s c", p=P
                    ),
                )
            toks.append(tokb)

        def emit_G_tile(gps, sb, k):
            # gps = [m0 tile (cols 0:512), m1 tile (cols 0:384 = G cols
            # 128:512), m2 tile (cols 0:256 = G cols 256:512)]; the m3
            # [128,128] block gets its own accumulation pass later (a PSUM
            # bank supports only one open accumulation group at a time).
            st, sp = (k == 0), (k == NT - 1)
            nc.tensor.matmul(gps[0][:, 0:C], sb[:, 0:P], sb[:, 0:C],
                             start=st, stop=sp)
            nc.tensor.matmul(gps[1][:, 0:C - P], sb[:, P:2 * P], sb[:, P:C],
                             start=st, stop=sp)
            nc.tensor.matmul(gps[2][:, 0:2 * P], sb[:, 2 * P:3 * P],
                             sb[:, 2 * P:C], start=st, stop=sp)

        def emit_G3(toks):
            """deferred G(3,3) block: one [128,128] accumulation over all
            32 n-tiles, in a psO slot (runs after W1 frees the ring)."""
            g3 = psO.tile([P, P], F32, tag="o", name="g3")
            for k in range(NT):
                sb = toks[k // 4][:, (k % 4) * C + 3 * P:(k % 4) * C + C]
                nc.tensor.matmul(g3[:], sb[:], sb[:],
                                 start=(k == 0), stop=(k == NT - 1))
            return g3

        def emit_T_tile(dst3, sb, k, ncol):
            """transpose [128n, 512c] sub-tile k into dst3 [p, m, ncol] view"""
            tps = psO.tile([P, C], F16, tag="o", name="tps")
            for m in range(CB):
                nc.tensor.transpose(
                    tps[:, m * P:(m + 1) * P], sb[:, m * P:(m + 1) * P], ident[:],
                )
            kk = k % (ncol // P)
            dst = dst3[:, :, kk * P:(kk + 1) * P]
            src = tps[:].rearrange("p (m n) -> p m n", m=CB)
            if k % 2:
                nc.scalar.copy(dst, src)
            else:
                nc.vector.tensor_copy(dst, src)

        def out_chunk(rhs_fn, BDs, kb, odram, ptag):
            """outT for 512 n-cols (tile-group kb): per pair p one matmul
            [c-block p, 512 n]; drain (split DVE/ACT) and write per pair."""
            ost = ostp.tile([P, 4 * 512], F16, tag="ost", name="ost")
            for p in range(CB):
                ops = psG.tile([P, 512], F32, tag=ptag, name=f"ops{p}")
                nc.tensor.matmul(ops[:], BDs[p][:], rhs_fn(p),
                                 start=True, stop=True)
                if p % 2:
                    nc.scalar.copy(ost[:, p * 512:(p + 1) * 512], ops[:])
                else:
                    nc.vector.tensor_copy(ost[:, p * 512:(p + 1) * 512], ops[:])
                # issue from the (otherwise idle) sync engine: a trigger that
                # waits on a drain must not block the drain engines' queues
                nc.sync.dma_start(
                    odram[kb, :, p, :], ost[:, p * 512:(p + 1) * 512])

        def chain_drain_mirror(gps, g3, vec_drain=False):
            """G psum -> g_sb fp16 with mirrored lower blocks."""
            g_sb = [gqp.tile([P, C], F16, tag="gq", name=f"g{m}")
                    for m in range(CB)]
            cp = nc.vector.tensor_copy if vec_drain else nc.scalar.copy
            cp(g_sb[0][:, 0:C], gps[0][:, 0:C])
            cp(g_sb[1][:, P:C], gps[1][:, 0:C - P])
            cp(g_sb[2][:, 2 * P:C], gps[2][:, 0:2 * P])
            cp(g_sb[3][:, 3 * P:C], g3[:])
            nmir = 0
            for i in range(CB):
                for j in range(G_OFF[i] // P):
                    mps = psO.tile([P, P], F16, tag="o", name="mps")
                    nc.tensor.transpose(
                        mps[:], g_sb[j][:, i * P:(i + 1) * P], ident[:],
                    )
                    if nmir % 2:
                        nc.scalar.copy(g_sb[i][:, j * P:(j + 1) * P], mps[:])
                    else:
                        nc.vector.tensor_copy(
                            g_sb[i][:, j * P:(j + 1) * P], mps[:])
                    nmir += 1
            return g_sb

        def chain_Q(g_sb, w, i):
            qp = psO.tile([P, C], F32, tag="o", name=f"qp{i}")
            for j in range(CB):
                nc.tensor.matmul(
                    qp[:], g_sb[j][:, i * P:(i + 1) * P],
                    wchunk(w, j)[:, 0:C], start=(j == 0), stop=(j == 3),
                )
            q = gqp.tile([P, C], F16, tag="gq", name=f"q{i}")
            nc.scalar.copy(q[:], qp[:])
            return q

        def chain_ctx(q_sb, w, p):
            """ctx psum for pair p -> softmax -> normalized F tile (fp16)."""
            cps = psO.tile([P, P], F32, tag="o", name=f"cps{p}")
            for j in range(CB):
                nc.tensor.matmul(
                    cps[:],
                    wchunk(w, j)[:, C + p * P:C + (p + 1) * P],
                    q_sb[j][:, p * P:(p + 1) * P],
                    start=(j == 0), stop=(j == 3),
                )
            nm = smallp.tile([P, 1], F32, tag="nm", name="nm")
            sm = smallp.tile([P, 1], F32, tag="sm", name="sm")
            rv = smallp.tile([P, 1], F32, tag="rv", name="rv")
            pp = smallp.tile([P, D], F32, tag="pp", name="pp")
            fp = fpool.tile([P, P], F16, tag="F", name="fp")
            nc.gpsimd.memset(fp[:], 0.0)
            for dd in range(2):
                s0 = slice(dd * D, (dd + 1) * D)
                blk = cps[s0, s0]
                nc.vector.reduce_max(nm[s0, :], blk, axis=AX.X, negate=True)
                nc.scalar.activation(
                    pp[s0, :], blk, ACT_EXP, bias=nm[s0, :], scale=1.0,
                    accum_out=sm[s0, :],
                )
            nc.vector.reciprocal(rv[:], sm[:])
            for dd in range(2):
                s0 = slice(dd * D, (dd + 1) * D)
                nc.vector.tensor_scalar_mul(fp[s0, s0], pp[s0, :], rv[s0, :])
            return fp

        def chain_bd(Fs):
            BDs = []
            for p in range(CB):
                bps = psO.tile([P, P], F16, tag="o", name="bps")
                nc.tensor.transpose(bps[:], Fs[p][:], ident[:])
                bd = bdpool.tile([P, P], F16, tag="bd", name=f"bd{p}")
                nc.vector.tensor_copy(bd[:], bps[:])
                BDs.append(bd)
            return BDs

        # ---- schedule ----
        # W1: joint G_x+G_b accumulation + T_x on PE; T_b via DMA crossbar
        toks_x, toks_b = [], []
        load_tok(toks_x, xb, 0, split=True)
        load_tok(toks_b, bb, 0, split=True)
        for kb in range(NBIG):
            if kb > 0:
                load_tok(toks_x, xb, kb)
                load_tok(toks_b, bb, kb)
            if kb == 3:
                load_weights()
        for kb in range(XBAR_KB):
            nc.sync.dma_start_transpose(
                xT_b[kb][:].rearrange("p (m n) -> p m n", m=CB),
                bb[kb * 512:(kb + 1) * 512, :],
            )

        gx = [psG.tile([P, C], F32, tag="g", name=f"gx{m}") for m in range(3)]
        gb = [psG.tile([P, C], F32, tag="g", name=f"gb{m}") for m in range(3)]
        for kb in range(NBIG):
            for sub in range(4):
                k = kb * 4 + sub
                sx = toks_x[kb][:, sub * C:(sub + 1) * C]
                sb_ = toks_b[kb][:, sub * C:(sub + 1) * C]
                emit_G_tile(gx, sx, k)
                emit_G_tile(gb, sb_, k)
                emit_T_tile(
                    xT_x[:].rearrange("p (m n) -> p m n", m=CB), sx, k, N)
        g3x = emit_G3(toks_x)

        # chain A (x's ctx -> BD1); fill the softmax bubble with T_b(last kb)
        # and the deferred G_b(3,3) accumulation
        g1 = chain_drain_mirror(gx, g3x)
        q1 = [chain_Q(g1, w_x, i) for i in range(CB)]
        Fs1 = [chain_ctx(q1, w_x, p) for p in range(CB)]
        for sub in range(4):
            k = XBAR_KB * 4 + sub
            sb_ = toks_b[XBAR_KB][:, sub * C:(sub + 1) * C]
            emit_T_tile(
                xT_b[XBAR_KB][:].rearrange("p (m n) -> p m n", m=CB),
                sb_, k, 512)
        # pre-drain g3b to SBUF: its psO slot must be recyclable before
        # chain B (which runs much later) reads the value
        g3b_ps = emit_G3(toks_b)
        g3b = g3p.tile([P, P], F16, tag="g3", name="g3b")
        nc.scalar.copy(g3b[:], g3b_ps[:])
        bd1 = chain_bd(Fs1)

        # production-B: out_b chunks interleaved with chain B pieces so the
        # ob writes overlap chain B's PE work
        pieces = []
        g2_box, q2_box, Fs2_box = [], [], []
        pieces.append(lambda: g2_box.extend(chain_drain_mirror(gb, g3b, True)))
        for i in range(CB):
            pieces.append(lambda i=i: q2_box.append(chain_Q(g2_box, w_b, i)))
        for p in range(CB):
            pieces.append(lambda p=p: Fs2_box.append(chain_ctx(q2_box, w_b, p)))

        for kb in range(NBIG):
            out_chunk(lambda p, kb=kb: xT_b[kb][:, p * 512:(p + 1) * 512],
                      bd1, kb, ob, "g")
            if pieces:
                pieces.pop(0)()
            if kb == 0 and pieces:
                pieces.pop(0)()
        while pieces:
            pieces.pop(0)()
        bd2 = chain_bd(Fs2_box)

        # production-X
        for kb in range(NBIG):
            out_chunk(
                lambda p, kb=kb: xT_x[:, p * N + kb * 512:p * N + (kb + 1) * 512],
                bd2, kb, ox, "g")


_NC_CACHE = None


def _get_nc():
    global _NC_CACHE
    if _NC_CACHE is None:
        _NC_CACHE = build_nc()
    return _NC_CACHE


def _prep_inputs(x, blood, W1, W2):
    x16 = np.ascontiguousarray(np.asarray(x, dtype=np.float32).astype(np.float16))
    b16 = np.ascontiguousarray(
        np.asarray(blood, dtype=np.float32).astype(np.float16))
    w1t = np.ascontiguousarray(np.asarray(W1, dtype=np.float32).T)
    w2t = np.ascontiguousarray(np.asarray(W2, dtype=np.float32).T)
    w1t[:, :C] *= SCALE  # fold softmax scale into the k-projection (exact: 2^-3)
    w2t[:, :C] *= SCALE
    w1t = w1t.astype(np.float16)
    w2t = w2t.astype(np.float16)
    return [
        {"xb": x16[b], "bb": b16[b], "w1t": w1t, "w2t": w2t} for b in range(B)
    ]


def _unshuffle(arr):
    """[NBIG, P, CB, 512] blocked-transposed fp16 -> [N, C] f32 natural."""
    # arr[kb, part, p, col] = out[kb*512 + col, p*128 + part]
    return np.ascontiguousarray(
        arr.transpose(0, 3, 2, 1).reshape(N, C).astype(np.float32))


def kernel(x, blood, W1, W2, trace=False):
    nc = _get_nc()
    in_maps = _prep_inputs(x, blood, W1, W2)
    res = run_bass_kernel_spmd(nc, in_maps, core_ids=list(range(B)), trace=trace)
    out_x = np.stack([_unshuffle(res.results[b]["oxT"]) for b in range(B)])
    out_b = np.stack([_unshuffle(res.results[b]["obT"]) for b in range(B)])
    if trace:
        kernel.last_results = res
    return (out_x, out_b)


# revision 20
# speedup vs baseline: 1.1405x; 1.1405x over previous
"""Trainium2 Bass kernel for nn_CrossAttention_31791347925417.

Math (per batch b, per stream tok in {x, blood} with weight W in {W1, W2}):
    kv = tok @ W.T ; k, v heads [H, N, D]
    ctx = softmax_d( SCALE * k_h^T v_h )          # [H, D, D], softmax over first D
    out_x = x_h @ ctx2_h ; out_b = blood_h @ ctx1_h

Refactor (Gram trick):
    k_h^T v_h = W_k_h (tok^T tok) W_v_h^T  with G = tok^T tok  [C, C]
so the N=4096 contraction happens once (G), and everything downstream is tiny
[C,C]-scale work.  ctxT blocks come from Q = G @ WkT then per head-pair
WvT_pair^T @ Q_pair whose diagonal 64x64 blocks are ctxT_h.  Softmax along the
free axis; normalized probs land in the diagonal blocks of a zeroed [128,128]
tile F; BD = F^T (PE transpose) is the block-diagonal ctx pair used by
    out[n, (h,e)] = sum_{(h,d)} tokT[(h,d), n] * BD[(h,d), (h,e)]

All on-chip data is fp16 (PSUM accumulation stays f32): fp16 matmuls and
transposes run at 1 cycle/column on the PE, and fp16 HBM I/O halves DMA
traffic vs f32.  Host casts inputs to fp16 and upcasts the fp16 outputs;
SCALE (2^-3, exact) folds into the k-weights.

Schedule (v3): ONE joint accumulation phase computes G_x and G_b together
(6 PSUM banks: per stream m0, m1, m2+m3 packed) plus the x transposes on PE;
blood transposes go through the DMA crossbar (dma_start_transpose) straight
from DRAM into per-group SBUF tiles, off the PE's critical path.  Chain A
runs, then out_b production is interleaved with chain B's matmuls so the ob
writes and chain B share the window; finally out_x streams out.  Both 4 MB
output streams thus pipeline against live compute instead of draining at the
end.

Sharding: data-parallel over batch B=8 across the 8 cores; weights replicated.
"""

import sys

if "/opt/trn_rl_repo" not in sys.path:
    sys.path.insert(0, "/opt/trn_rl_repo")

import numpy as np

from concourse import bacc, masks, mybir, tile
from concourse.bass_utils import run_bass_kernel_spmd

B, N, C, H = 8, 4096, 512, 8
D = C // H
SCALE = D ** -0.5
P = 128
NBIG = N // 512          # 8 big row tiles (512 rows each)
NT = N // P              # 32 n-tiles
CB = C // P              # 4 column blocks == head pairs
F32 = mybir.dt.float32
F16 = mybir.dt.float16
AX = mybir.AxisListType
ACT_EXP = mybir.ActivationFunctionType.Exp

# G is symmetric: row-block m only needs columns >= G_OFF[m]
G_OFF = [0, P, 2 * P, 3 * P]

def build_nc():
    nc = bacc.Bacc("TRN2", target_bir_lowering=False, debug=False)

    xb = nc.dram_tensor("xb", [N, C], F16, kind="ExternalInput").ap()
    bb = nc.dram_tensor("bb", [N, C], F16, kind="ExternalInput").ap()
    w1t = nc.dram_tensor("w1t", [C, 2 * C], F16, kind="ExternalInput").ap()
    w2t = nc.dram_tensor("w2t", [C, 2 * C], F16, kind="ExternalInput").ap()
    # blocked transposed output layout: [kb, part(c within pair), pair, n-col]
    ox = nc.dram_tensor("oxT", [NBIG, P, CB, 512], F16, kind="ExternalOutput").ap()
    ob = nc.dram_tensor("obT", [NBIG, P, CB, 512], F16, kind="ExternalOutput").ap()

    with tile.TileContext(nc) as tc:
        _emit(nc, tc, xb, bb, w1t, w2t, ox, ob)

    nc.compile()
    return nc


def _emit(nc, tc, xb, bb, w1t, w2t, ox, ob):
    from contextlib import ExitStack

    ctx = ExitStack()
    with ctx:
        const = ctx.enter_context(tc.tile_pool(name="const", bufs=1))
        wpool = ctx.enter_context(tc.tile_pool(name="wpool", bufs=1))
        tokp = ctx.enter_context(tc.tile_pool(name="tokp", bufs=16))
        xtp = ctx.enter_context(tc.tile_pool(name="xtp", bufs=NBIG))
        xtbp = ctx.enter_context(tc.tile_pool(name="xtbp", bufs=NBIG))
        gqp = ctx.enter_context(tc.tile_pool(name="gqp", bufs=8))
        smallp = ctx.enter_context(tc.tile_pool(name="smallp", bufs=2))
        fpool = ctx.enter_context(tc.tile_pool(name="fpool", bufs=8))
        bdpool = ctx.enter_context(tc.tile_pool(name="bdpool", bufs=8))
        ostp = ctx.enter_context(tc.tile_pool(name="ostp", bufs=3))
        # PSUM: 3 banks G accumulators (x's slots are drained before blood's
        # allocations claim them), 3 banks out-psum rotation, 2 banks for
        # transposes / chain work.  3+3+2 = all 8 banks.
        psGa = ctx.enter_context(tc.tile_pool(name="psGa", bufs=3, space="PSUM"))
        psOut = ctx.enter_context(tc.tile_pool(name="psOut", bufs=3, space="PSUM"))
        psO = ctx.enter_context(tc.tile_pool(name="psO", bufs=2, space="PSUM"))

        ident = const.tile([P, P], F16, tag="idh")
        masks.make_identity(nc, ident[:])

        # weights: chunk j (c-rows 128j..128j+128) lives at cols [j*2C, (j+1)*2C)
        w_x = wpool.tile([P, CB * 2 * C], F16, tag="wx")
        w_b = wpool.tile([P, CB * 2 * C], F16, tag="wb")

        def load_weights():
            nc.sync.dma_start(
                w_x[:].rearrange("p (j c) -> p j c", j=CB),
                w1t[:, :].rearrange("(j p) c -> p j c", p=P),
            )
            nc.sync.dma_start(
                w_b[:].rearrange("p (j c) -> p j c", j=CB),
                w2t[:, :].rearrange("(j p) c -> p j c", p=P),
            )

        def wchunk(w, j):
            return w[:, j * 2 * C:(j + 1) * 2 * C]

        # transposed tokens: per-kb contiguous tiles (the xbar DMA transpose
        # needs a contiguous destination), pair block m at cols [m*512, ...)
        xT_x = [xtp.tile([P, CB * 512], F16, tag="xtx", name=f"xtx{kb}")
                for kb in range(NBIG)]
        xT_b = [xtbp.tile([P, CB * 512], F16, tag="xtb", name=f"xtb{kb}")
                for kb in range(NBIG)]

        def load_tok(toks, tok_dram, kb, split=False):
            tokb = tokp.tile([P, 4 * C], F16, tag="tok", name=f"tok{kb}")
            if split:
                for sub in range(4):
                    nc.sync.dma_start(
                        tokb[:, sub * C:(sub + 1) * C],
                        tok_dram[kb * 512 + sub * P:kb * 512 + (sub + 1) * P, :],
                    )
            else:
                nc.sync.dma_start(
                    tokb[:].rearrange("p (s c) -> p s c", s=4),
                    tok_dram[kb * 512:(kb + 1) * 512, :].rearrange(
                        "(s p) c -> p s c", p=P
                    ),
                )
            toks.append(tokb)

        def emit_G_tile(gps, sb, k):
            # gps = [m0 tile (cols 0:512), m1 tile (cols 0:384 = G cols
            # 128:512), m2 tile (cols 0:256 = G cols 256:512)]; the m3
            # [128,128] block gets its own accumulation pass later (a PSUM
            # bank supports only one open accumulation group at a time).
            st, sp = (k == 0), (k == NT - 1)
            nc.tensor.matmul(gps[0][:, 0:C], sb[:, 0:P], sb[:, 0:C],
                             start=st, stop=sp)
            nc.tensor.matmul(gps[1][:, 0:C - P], sb[:, P:2 * P], sb[:, P:C],
                             start=st, stop=sp)
            nc.tensor.matmul(gps[2][:, 0:2 * P], sb[:, 2 * P:3 * P],
                             sb[:, 2 * P:C], start=st, stop=sp)

        def emit_G3(toks):
            """deferred G(3,3) block: one [128,128] accumulation over all
            32 n-tiles, in a psO slot (runs after W1 frees the ring)."""
            g3 = psO.tile([P, P], F32, tag="o", name="g3")
            for k in range(NT):
                sb = toks[k // 4][:, (k % 4) * C + 3 * P:(k % 4) * C + C]
                nc.tensor.matmul(g3[:], sb[:], sb[:],
                                 start=(k == 0), stop=(k == NT - 1))
            return g3

        def emit_T_tile(dst3, sb, k, ncol):
            """transpose [128n, 512c] sub-tile k into dst3 [p, m, ncol] view"""
            tps = psO.tile([P, C], F16, tag="o", name="tps")
            for m in range(CB):
                nc.tensor.transpose(
                    tps[:, m * P:(m + 1) * P], sb[:, m * P:(m + 1) * P], ident[:],
                )
            kk = k % (ncol // P)
            dst = dst3[:, :, kk * P:(kk + 1) * P]
            src = tps[:].rearrange("p (m n) -> p m n", m=CB)
            if k % 2:
                nc.scalar.copy(dst, src)
            else:
                nc.vector.tensor_copy(dst, src)

        def out_chunk(rhs_fn, BDs, kb, odram, ptag):
            """outT for 512 n-cols (tile-group kb): per pair p one matmul
            [c-block p, 512 n]; drain (split DVE/ACT), one batched write per
            kb with the issuing engine alternating so a trigger waiting on
            the other engine's drain never blocks a queue for long."""
            ost = ostp.tile([P, 4 * 512], F16, tag="ost", name="ost")
            for p in range(CB):
                ops = psOut.tile([P, 512], F32, tag=ptag, name=f"ops{p}")
                nc.tensor.matmul(ops[:], BDs[p][:], rhs_fn(p),
                                 start=True, stop=True)
                if p % 2:
                    nc.scalar.copy(ost[:, p * 512:(p + 1) * 512], ops[:])
                else:
                    nc.vector.tensor_copy(ost[:, p * 512:(p + 1) * 512], ops[:])
            eng = nc.scalar if kb % 2 else nc.sync
            eng.dma_start(odram[kb], ost[:].rearrange("p (c n) -> p c n", c=CB))

        def chain_drain_mirror(gps, g3, vec_drain=False):
            """G psum -> g_sb fp16 with mirrored lower blocks."""
            g_sb = [gqp.tile([P, C], F16, tag="gq", name=f"g{m}")
                    for m in range(CB)]
            cp = nc.vector.tensor_copy if vec_drain else nc.scalar.copy
            cp(g_sb[0][:, 0:C], gps[0][:, 0:C])
            cp(g_sb[1][:, P:C], gps[1][:, 0:C - P])
            cp(g_sb[2][:, 2 * P:C], gps[2][:, 0:2 * P])
            cp(g_sb[3][:, 3 * P:C], g3[:])
            nmir = 0
            for i in range(CB):
                for j in range(G_OFF[i] // P):
                    mps = psO.tile([P, P], F16, tag="o", name="mps")
                    nc.tensor.transpose(
                        mps[:], g_sb[j][:, i * P:(i + 1) * P], ident[:],
                    )
                    if nmir % 2:
                        nc.scalar.copy(g_sb[i][:, j * P:(j + 1) * P], mps[:])
                    else:
                        nc.vector.tensor_copy(
                            g_sb[i][:, j * P:(j + 1) * P], mps[:])
                    nmir += 1
            return g_sb

        def chain_Q(g_sb, w, i):
            qp = psO.tile([P, C], F32, tag="o", name=f"qp{i}")
            for j in range(CB):
                nc.tensor.matmul(
                    qp[:], g_sb[j][:, i * P:(i + 1) * P],
                    wchunk(w, j)[:, 0:C], start=(j == 0), stop=(j == 3),
                )
            q = gqp.tile([P, C], F16, tag="gq", name=f"q{i}")
            nc.scalar.copy(q[:], qp[:])
            return q

        def chain_ctx(q_sb, w, p):
            """ctx psum for pair p -> softmax -> normalized F tile (fp16)."""
            cps = psO.tile([P, P], F32, tag="o", name=f"cps{p}")
            for j in range(CB):
                nc.tensor.matmul(
                    cps[:],
                    wchunk(w, j)[:, C + p * P:C + (p + 1) * P],
                    q_sb[j][:, p * P:(p + 1) * P],
                    start=(j == 0), stop=(j == 3),
                )
            nm = smallp.tile([P, 1], F32, tag="nm", name="nm")
            sm = smallp.tile([P, 1], F32, tag="sm", name="sm")
            rv = smallp.tile([P, 1], F32, tag="rv", name="rv")
            pp = smallp.tile([P, D], F32, tag="pp", name="pp")
            fp = fpool.tile([P, P], F16, tag="F", name="fp")
            nc.gpsimd.memset(fp[:], 0.0)
            for dd in range(2):
                s0 = slice(dd * D, (dd + 1) * D)
                blk = cps[s0, s0]
                nc.vector.reduce_max(nm[s0, :], blk, axis=AX.X, negate=True)
                nc.scalar.activation(
                    pp[s0, :], blk, ACT_EXP, bias=nm[s0, :], scale=1.0,
                    accum_out=sm[s0, :],
                )
            nc.vector.reciprocal(rv[:], sm[:])
            for dd in range(2):
                s0 = slice(dd * D, (dd + 1) * D)
                nc.vector.tensor_scalar_mul(fp[s0, s0], pp[s0, :], rv[s0, :])
            return fp

        def chain_bd(Fs):
            BDs = []
            for p in range(CB):
                bps = psO.tile([P, P], F16, tag="o", name="bps")
                nc.tensor.transpose(bps[:], Fs[p][:], ident[:])
                bd = bdpool.tile([P, P], F16, tag="bd", name=f"bd{p}")
                nc.vector.tensor_copy(bd[:], bps[:])
                BDs.append(bd)
            return BDs

        # ---- schedule ----
        # loads first (they own the DMA queues early), then the x-stream
        # crossbar transposes (queued behind the loads)
        toks_x, toks_b = [], []
        load_tok(toks_x, xb, 0, split=True)
        for kb in range(1, NBIG):
            load_tok(toks_x, xb, kb)
        load_weights()
        for kb in range(NBIG):
            load_tok(toks_b, bb, kb)
        for kb in range(NBIG):
            nc.sync.dma_start_transpose(
                xT_x[kb][:].rearrange("p (m n) -> p m n", m=CB),
                xb[kb * 512:(kb + 1) * 512, :],
            )

        # phase A: G_x accumulation only (x transposes ride the crossbar)
        gx = [psGa.tile([P, C], F32, tag="g", name=f"gx{m}") for m in range(3)]
        for kb in range(NBIG):
            for sub in range(4):
                emit_G_tile(gx, toks_x[kb][:, sub * C:(sub + 1) * C],
                            kb * 4 + sub)
        g3x = emit_G3(toks_x)

        # chain A (x's ctx -> BD1); fill the softmax bubble with the first
        # blood tile-group's transposes + G_b start
        g1 = chain_drain_mirror(gx, g3x)
        q1 = [chain_Q(g1, w_x, i) for i in range(CB)]
        Fs1 = [chain_ctx(q1, w_x, p) for p in range(CB)]
        gb = [psGa.tile([P, C], F32, tag="g", name=f"gb{m}") for m in range(3)]
        for sub in range(4):
            sb_ = toks_b[0][:, sub * C:(sub + 1) * C]
            emit_T_tile(
                xT_b[0][:].rearrange("p (m n) -> p m n", m=CB), sb_, sub, 512)
            emit_G_tile(gb, sb_, sub)
        bd1 = chain_bd(Fs1)

        # phase B: per 512-row group: transpose blood, accumulate G_b, and
        # produce out_b for the previous group (writes stream out early)
        for kb in range(1, NBIG):
            for sub in range(4):
                k = kb * 4 + sub
                sb_ = toks_b[kb][:, sub * C:(sub + 1) * C]
                emit_T_tile(
                    xT_b[kb][:].rearrange("p (m n) -> p m n", m=CB),
                    sb_, k, 512)
                emit_G_tile(gb, sb_, k)
            out_chunk(lambda p, kb=kb - 1: xT_b[kb][:, p * 512:(p + 1) * 512],
                      bd1, kb - 1, ob, "g")
        g3b = emit_G3(toks_b)

        # chain B; fill its softmax bubble with the last out_b group
        g2 = chain_drain_mirror(gb, g3b, True)
        q2 = [chain_Q(g2, w_b, i) for i in range(CB)]
        Fs2 = [chain_ctx(q2, w_b, p) for p in range(CB)]
        out_chunk(lambda p: xT_b[NBIG - 1][:, p * 512:(p + 1) * 512],
                  bd1, NBIG - 1, ob, "g")
        bd2 = chain_bd(Fs2)

        # phase C: out_x production (~0.5 MB per group, pipelined writes)
        for kb in range(NBIG):
            out_chunk(
                lambda p, kb=kb: xT_x[kb][:, p * 512:(p + 1) * 512],
                bd2, kb, ox, "g")


_NC_CACHE = None


def _get_nc():
    global _NC_CACHE
    if _NC_CACHE is None:
        _NC_CACHE = build_nc()
    return _NC_CACHE


def _prep_inputs(x, blood, W1, W2):
    x16 = np.ascontiguousarray(np.asarray(x, dtype=np.float32).astype(np.float16))
    b16 = np.ascontiguousarray(
        np.asarray(blood, dtype=np.float32).astype(np.float16))
    w1t = np.ascontiguousarray(np.asarray(W1, dtype=np.float32).T)
    w2t = np.ascontiguousarray(np.asarray(W2, dtype=np.float32).T)
    w1t[:, :C] *= SCALE  # fold softmax scale into the k-projection (exact: 2^-3)
    w2t[:, :C] *= SCALE
    w1t = w1t.astype(np.float16)
    w2t = w2t.astype(np.float16)
    return [
        {"xb": x16[b], "bb": b16[b], "w1t": w1t, "w2t": w2t} for b in range(B)
    ]


def _unshuffle(arr):
    """[NBIG, P, CB, 512] blocked-transposed fp16 -> [N, C] f32 natural."""
    # arr[kb, part, p, col] = out[kb*512 + col, p*128 + part]
    return np.ascontiguousarray(
        arr.transpose(0, 3, 2, 1).reshape(N, C).astype(np.float32))


def kernel(x, blood, W1, W2, trace=False):
    nc = _get_nc()
    in_maps = _prep_inputs(x, blood, W1, W2)
    res = run_bass_kernel_spmd(nc, in_maps, core_ids=list(range(B)), trace=trace)
    out_x = np.stack([_unshuffle(res.results[b]["oxT"]) for b in range(B)])
    out_b = np.stack([_unshuffle(res.results[b]["obT"]) for b in range(B)])
    if trace:
        kernel.last_results = res
    return (out_x, out_b)


# revision 22
# speedup vs baseline: 1.2802x; 1.1225x over previous
"""Trainium2 Bass kernel for nn_CrossAttention_31791347925417.

Math (per batch b, per stream tok in {x, blood} with weight W in {W1, W2}):
    kv = tok @ W.T ; k, v heads [H, N, D]
    ctx = softmax_d( SCALE * k_h^T v_h )          # [H, D, D], softmax over first D
    out_x = x_h @ ctx2_h ; out_b = blood_h @ ctx1_h

Refactor (Gram trick):
    k_h^T v_h = W_k_h (tok^T tok) W_v_h^T  with G = tok^T tok  [C, C]
so the N=4096 contraction happens once (G), and everything downstream is tiny
[C,C]-scale work.  ctxT blocks come from Q = G @ WkT then per head-pair
WvT_pair^T @ Q_pair whose diagonal 64x64 blocks are ctxT_h.  Softmax along the
free axis; normalized probs land in the diagonal blocks of a zeroed [128,128]
tile F; BD = F^T (PE transpose) is the block-diagonal ctx pair used by
    out[n, (h,e)] = sum_{(h,d)} tokT[(h,d), n] * BD[(h,d), (h,e)]

All on-chip data is fp16 (PSUM accumulation stays f32): fp16 matmuls and
transposes run at 1 cycle/column on the PE, and fp16 HBM I/O halves DMA
traffic vs f32.  Host casts inputs to fp16 and upcasts the fp16 outputs;
SCALE (2^-3, exact) folds into the k-weights.

Schedule (v3): ONE joint accumulation phase computes G_x and G_b together
(6 PSUM banks: per stream m0, m1, m2+m3 packed) plus the x transposes on PE;
blood transposes go through the DMA crossbar (dma_start_transpose) straight
from DRAM into per-group SBUF tiles, off the PE's critical path.  Chain A
runs, then out_b production is interleaved with chain B's matmuls so the ob
writes and chain B share the window; finally out_x streams out.  Both 4 MB
output streams thus pipeline against live compute instead of draining at the
end.

Sharding: data-parallel over batch B=8 across the 8 cores; weights replicated.
"""

import sys

if "/opt/trn_rl_repo" not in sys.path:
    sys.path.insert(0, "/opt/trn_rl_repo")

import numpy as np

from concourse import bacc, masks, mybir, tile
from concourse.bass_utils import run_bass_kernel_spmd

B, N, C, H = 8, 4096, 512, 8
D = C // H
SCALE = D ** -0.5
P = 128
NBIG = N // 512          # 8 big row tiles (512 rows each)
NT = N // P              # 32 n-tiles
CB = C // P              # 4 column blocks == head pairs
F32 = mybir.dt.float32
F16 = mybir.dt.float16
AX = mybir.AxisListType
ACT_EXP = mybir.ActivationFunctionType.Exp

# G is symmetric: row-block m only needs columns >= G_OFF[m]
G_OFF = [0, P, 2 * P, 3 * P]

def build_nc():
    nc = bacc.Bacc("TRN2", target_bir_lowering=False, debug=False)

    xb = nc.dram_tensor("xb", [N, C], F16, kind="ExternalInput").ap()
    bb = nc.dram_tensor("bb", [N, C], F16, kind="ExternalInput").ap()
    w1t = nc.dram_tensor("w1t", [C, 2 * C], F16, kind="ExternalInput").ap()
    w2t = nc.dram_tensor("w2t", [C, 2 * C], F16, kind="ExternalInput").ap()
    # blocked transposed output layout: [kb, part(c within pair), pair, n-col]
    ox = nc.dram_tensor("oxT", [NBIG, P, CB, 512], F16, kind="ExternalOutput").ap()
    ob = nc.dram_tensor("obT", [NBIG, P, CB, 512], F16, kind="ExternalOutput").ap()

    with tile.TileContext(nc) as tc:
        _emit(nc, tc, xb, bb, w1t, w2t, ox, ob)

    nc.compile()
    return nc


def _emit(nc, tc, xb, bb, w1t, w2t, ox, ob):
    from contextlib import ExitStack

    ctx = ExitStack()
    with ctx:
        const = ctx.enter_context(tc.tile_pool(name="const", bufs=1))
        wpool = ctx.enter_context(tc.tile_pool(name="wpool", bufs=1))
        tokp = ctx.enter_context(tc.tile_pool(name="tokp", bufs=16))
        xtp = ctx.enter_context(tc.tile_pool(name="xtp", bufs=NBIG))
        xtbp = ctx.enter_context(tc.tile_pool(name="xtbp", bufs=NBIG))
        gqp = ctx.enter_context(tc.tile_pool(name="gqp", bufs=8))
        smallp = ctx.enter_context(tc.tile_pool(name="smallp", bufs=4))
        fpool = ctx.enter_context(tc.tile_pool(name="fpool", bufs=8))
        bdpool = ctx.enter_context(tc.tile_pool(name="bdpool", bufs=8))
        ostp = ctx.enter_context(tc.tile_pool(name="ostp", bufs=3))
        # PSUM: 3 banks G accumulators (x's slots are drained before blood's
        # allocations claim them), 3 banks out-psum rotation, 2 banks for
        # transposes / chain work.  3+3+2 = all 8 banks.
        psGa = ctx.enter_context(tc.tile_pool(name="psGa", bufs=3, space="PSUM"))
        psOut = ctx.enter_context(tc.tile_pool(name="psOut", bufs=3, space="PSUM"))
        psO = ctx.enter_context(tc.tile_pool(name="psO", bufs=2, space="PSUM"))

        ident = const.tile([P, P], F16, tag="idh")
        masks.make_identity(nc, ident[:])

        # weights: chunk j (c-rows 128j..128j+128) lives at cols [j*2C, (j+1)*2C)
        w_x = wpool.tile([P, CB * 2 * C], F16, tag="wx")
        w_b = wpool.tile([P, CB * 2 * C], F16, tag="wb")

        def load_weights():
            nc.sync.dma_start(
                w_x[:].rearrange("p (j c) -> p j c", j=CB),
                w1t[:, :].rearrange("(j p) c -> p j c", p=P),
            )
            nc.sync.dma_start(
                w_b[:].rearrange("p (j c) -> p j c", j=CB),
                w2t[:, :].rearrange("(j p) c -> p j c", p=P),
            )

        def wchunk(w, j):
            return w[:, j * 2 * C:(j + 1) * 2 * C]

        # transposed tokens: per-kb contiguous tiles (the xbar DMA transpose
        # needs a contiguous destination), pair block m at cols [m*512, ...)
        xT_x = [xtp.tile([P, CB * 512], F16, tag="xtx", name=f"xtx{kb}")
                for kb in range(NBIG)]
        xT_b = [xtbp.tile([P, CB * 512], F16, tag="xtb", name=f"xtb{kb}")
                for kb in range(NBIG)]

        def load_tok(toks, tok_dram, kb, split=False):
            tokb = tokp.tile([P, 4 * C], F16, tag="tok", name=f"tok{kb}")
            if split:
                for sub in range(4):
                    nc.sync.dma_start(
                        tokb[:, sub * C:(sub + 1) * C],
                        tok_dram[kb * 512 + sub * P:kb * 512 + (sub + 1) * P, :],
                    )
            else:
                nc.sync.dma_start(
                    tokb[:].rearrange("p (s c) -> p s c", s=4),
                    tok_dram[kb * 512:(kb + 1) * 512, :].rearrange(
                        "(s p) c -> p s c", p=P
                    ),
                )
            toks.append(tokb)

        def emit_G_tile(gps, sb, k):
            # gps = [m0 tile (cols 0:512), m1 tile (cols 0:384 = G cols
            # 128:512), m2 tile (cols 0:256 = G cols 256:512)]; the m3
            # [128,128] block gets its own accumulation pass later (a PSUM
            # bank supports only one open accumulation group at a time).
            st, sp = (k == 0), (k == NT - 1)
            nc.tensor.matmul(gps[0][:, 0:C], sb[:, 0:P], sb[:, 0:C],
                             start=st, stop=sp)
            nc.tensor.matmul(gps[1][:, 0:C - P], sb[:, P:2 * P], sb[:, P:C],
                             start=st, stop=sp)
            nc.tensor.matmul(gps[2][:, 0:2 * P], sb[:, 2 * P:3 * P],
                             sb[:, 2 * P:C], start=st, stop=sp)

        def emit_G3(toks):
            """deferred G(3,3) block: one [128,128] accumulation over all
            32 n-tiles, in a psO slot (runs after W1 frees the ring)."""
            g3 = psO.tile([P, P], F32, tag="o", name="g3")
            for k in range(NT):
                sb = toks[k // 4][:, (k % 4) * C + 3 * P:(k % 4) * C + C]
                nc.tensor.matmul(g3[:], sb[:], sb[:],
                                 start=(k == 0), stop=(k == NT - 1))
            return g3

        def emit_T_tile(dst3, sb, k, ncol):
            """transpose [128n, 512c] sub-tile k into dst3 [p, m, ncol] view"""
            tps = psO.tile([P, C], F16, tag="o", name="tps")
            for m in range(CB):
                nc.tensor.transpose(
                    tps[:, m * P:(m + 1) * P], sb[:, m * P:(m + 1) * P], ident[:],
                )
            kk = k % (ncol // P)
            dst = dst3[:, :, kk * P:(kk + 1) * P]
            src = tps[:].rearrange("p (m n) -> p m n", m=CB)
            if k % 2:
                nc.scalar.copy(dst, src)
            else:
                nc.vector.tensor_copy(dst, src)

        def out_chunk(rhs_fn, BDs, kb, odram, ptag):
            """outT for 512 n-cols (tile-group kb): per pair p one matmul
            [c-block p, 512 n]; drain (split DVE/ACT), one batched write per
            kb with the issuing engine alternating so a trigger waiting on
            the other engine's drain never blocks a queue for long."""
            ost = ostp.tile([P, 4 * 512], F16, tag="ost", name="ost")
            for p in range(CB):
                ops = psOut.tile([P, 512], F32, tag=ptag, name=f"ops{p}")
                nc.tensor.matmul(ops[:], BDs[p][:], rhs_fn(p),
                                 start=True, stop=True)
                if p % 2:
                    nc.scalar.copy(ost[:, p * 512:(p + 1) * 512], ops[:])
                else:
                    nc.vector.tensor_copy(ost[:, p * 512:(p + 1) * 512], ops[:])
            eng = nc.scalar if kb % 2 else nc.sync
            eng.dma_start(odram[kb], ost[:].rearrange("p (c n) -> p c n", c=CB))

        def chain_drain_mirror(gps, g3, vec_drain=False):
            """G psum -> g_sb fp16 with mirrored lower blocks."""
            g_sb = [gqp.tile([P, C], F16, tag="gq", name=f"g{m}")
                    for m in range(CB)]
            cp = nc.vector.tensor_copy if vec_drain else nc.scalar.copy
            cp(g_sb[0][:, 0:C], gps[0][:, 0:C])
            cp(g_sb[1][:, P:C], gps[1][:, 0:C - P])
            cp(g_sb[2][:, 2 * P:C], gps[2][:, 0:2 * P])
            cp(g_sb[3][:, 3 * P:C], g3[:])
            nmir = 0
            for i in range(CB):
                for j in range(G_OFF[i] // P):
                    mps = psO.tile([P, P], F16, tag="o", name="mps")
                    nc.tensor.transpose(
                        mps[:], g_sb[j][:, i * P:(i + 1) * P], ident[:],
                    )
                    if nmir % 2:
                        nc.scalar.copy(g_sb[i][:, j * P:(j + 1) * P], mps[:])
                    else:
                        nc.vector.tensor_copy(
                            g_sb[i][:, j * P:(j + 1) * P], mps[:])
                    nmir += 1
            return g_sb

        def chain_Q(g_sb, w, i):
            qp = psO.tile([P, C], F32, tag="o", name=f"qp{i}")
            for j in range(CB):
                nc.tensor.matmul(
                    qp[:], g_sb[j][:, i * P:(i + 1) * P],
                    wchunk(w, j)[:, 0:C], start=(j == 0), stop=(j == 3),
                )
            q = gqp.tile([P, C], F16, tag="gq", name=f"q{i}")
            nc.scalar.copy(q[:], qp[:])
            return q

        def chain_ctx(q_sb, w, p):
            """ctx psum for pair p -> softmax -> normalized F tile (fp16)."""
            cps = psO.tile([P, P], F32, tag="o", name=f"cps{p}")
            for j in range(CB):
                nc.tensor.matmul(
                    cps[:],
                    wchunk(w, j)[:, C + p * P:C + (p + 1) * P],
                    q_sb[j][:, p * P:(p + 1) * P],
                    start=(j == 0), stop=(j == 3),
                )
            nm = smallp.tile([P, 1], F32, tag="nm", name="nm")
            sm = smallp.tile([P, 1], F32, tag="sm", name="sm")
            rv = smallp.tile([P, 1], F32, tag="rv", name="rv")
            pp = smallp.tile([P, D], F32, tag="pp", name="pp")
            fp = fpool.tile([P, P], F16, tag="F", name="fp")
            nc.gpsimd.memset(fp[:], 0.0)
            for dd in range(2):
                s0 = slice(dd * D, (dd + 1) * D)
                blk = cps[s0, s0]
                nc.vector.reduce_max(nm[s0, :], blk, axis=AX.X, negate=True)
                nc.scalar.activation(
                    pp[s0, :], blk, ACT_EXP, bias=nm[s0, :], scale=1.0,
                    accum_out=sm[s0, :],
                )
            nc.vector.reciprocal(rv[:], sm[:])
            for dd in range(2):
                s0 = slice(dd * D, (dd + 1) * D)
                nc.vector.tensor_scalar_mul(fp[s0, s0], pp[s0, :], rv[s0, :])
            return fp

        def chain_bd(Fs):
            BDs = []
            for p in range(CB):
                bps = psO.tile([P, P], F16, tag="o", name="bps")
                nc.tensor.transpose(bps[:], Fs[p][:], ident[:])
                bd = bdpool.tile([P, P], F16, tag="bd", name=f"bd{p}")
                nc.vector.tensor_copy(bd[:], bps[:])
                BDs.append(bd)
            return BDs

        # ---- schedule ----
        # loads first (they own the DMA queues early), then the x-stream
        # crossbar transposes (queued behind the loads)
        toks_x, toks_b = [], []
        load_tok(toks_x, xb, 0, split=True)
        for kb in range(1, NBIG):
            load_tok(toks_x, xb, kb)
        load_weights()
        for kb in range(NBIG):
            load_tok(toks_b, bb, kb)

        # phase A: G_x accumulation + x transposes
        gx = [psGa.tile([P, C], F32, tag="g", name=f"gx{m}") for m in range(3)]
        for kb in range(NBIG):
            for sub in range(4):
                k = kb * 4 + sub
                sx = toks_x[kb][:, sub * C:(sub + 1) * C]
                emit_G_tile(gx, sx, k)
                emit_T_tile(
                    xT_x[kb][:].rearrange("p (m n) -> p m n", m=CB),
                    sx, k, 512)
        g3x = emit_G3(toks_x)

        # chain A (x's ctx -> BD1); fill the softmax bubble with the first
        # blood tile-group's transposes + G_b start
        g1 = chain_drain_mirror(gx, g3x)
        q1 = [chain_Q(g1, w_x, i) for i in range(CB)]
        Fs1 = [chain_ctx(q1, w_x, p) for p in range(CB)]
        gb = [psGa.tile([P, C], F32, tag="g", name=f"gb{m}") for m in range(3)]
        for sub in range(4):
            sb_ = toks_b[0][:, sub * C:(sub + 1) * C]
            emit_T_tile(
                xT_b[0][:].rearrange("p (m n) -> p m n", m=CB), sb_, sub, 512)
            emit_G_tile(gb, sb_, sub)
        bd1 = chain_bd(Fs1)

        # phase B: per 512-row group: transpose blood, accumulate G_b, and
        # produce out_b for the previous group (writes stream out early)
        for kb in range(1, NBIG):
            for sub in range(4):
                k = kb * 4 + sub
                sb_ = toks_b[kb][:, sub * C:(sub + 1) * C]
                emit_T_tile(
                    xT_b[kb][:].rearrange("p (m n) -> p m n", m=CB),
                    sb_, k, 512)
                emit_G_tile(gb, sb_, k)
            out_chunk(lambda p, kb=kb - 1: xT_b[kb][:, p * 512:(p + 1) * 512],
                      bd1, kb - 1, ob, "g")
        g3b = emit_G3(toks_b)

        # chain B; fill its softmax bubble with the last out_b group
        g2 = chain_drain_mirror(gb, g3b, True)
        q2 = [chain_Q(g2, w_b, i) for i in range(CB)]
        Fs2 = [chain_ctx(q2, w_b, p) for p in range(CB)]
        out_chunk(lambda p: xT_b[NBIG - 1][:, p * 512:(p + 1) * 512],
                  bd1, NBIG - 1, ob, "g")
        bd2 = chain_bd(Fs2)

        # phase C: out_x production (~0.5 MB per group, pipelined writes)
        for kb in range(NBIG):
            out_chunk(
                lambda p, kb=kb: xT_x[kb][:, p * 512:(p + 1) * 512],
                bd2, kb, ox, "g")


_NC_CACHE = None


def _get_nc():
    global _NC_CACHE
    if _NC_CACHE is None:
        _NC_CACHE = build_nc()
    return _NC_CACHE


def _prep_inputs(x, blood, W1, W2):
    x16 = np.ascontiguousarray(np.asarray(x, dtype=np.float32).astype(np.float16))
    b16 = np.ascontiguousarray(
        np.asarray(blood, dtype=np.float32).astype(np.float16))
    w1t = np.ascontiguousarray(np.asarray(W1, dtype=np.float32).T)
    w2t = np.ascontiguousarray(np.asarray(W2, dtype=np.float32).T)
    w1t[:, :C] *= SCALE  # fold softmax scale into the k-projection (exact: 2^-3)
    w2t[:, :C] *= SCALE
    w1t = w1t.astype(np.float16)
    w2t = w2t.astype(np.float16)
    return [
        {"xb": x16[b], "bb": b16[b], "w1t": w1t, "w2t": w2t} for b in range(B)
    ]


def _unshuffle(arr):
    """[NBIG, P, CB, 512] blocked-transposed fp16 -> [N, C] f32 natural."""
    # arr[kb, part, p, col] = out[kb*512 + col, p*128 + part]
    return np.ascontiguousarray(
        arr.transpose(0, 3, 2, 1).reshape(N, C).astype(np.float32))


def kernel(x, blood, W1, W2, trace=False):
    nc = _get_nc()
    in_maps = _prep_inputs(x, blood, W1, W2)
    res = run_bass_kernel_spmd(nc, in_maps, core_ids=list(range(B)), trace=trace)
    out_x = np.stack([_unshuffle(res.results[b]["oxT"]) for b in range(B)])
    out_b = np.stack([_unshuffle(res.results[b]["obT"]) for b in range(B)])
    if trace:
        kernel.last_results = res
    return (out_x, out_b)


# revision 25
# speedup vs baseline: 1.3292x; 1.0383x over previous
"""Trainium2 Bass kernel for nn_CrossAttention_31791347925417.

Math (per batch b, per stream tok in {x, blood} with weight W in {W1, W2}):
    kv = tok @ W.T ; k, v heads [H, N, D]
    ctx = softmax_d( SCALE * k_h^T v_h )          # [H, D, D], softmax over first D
    out_x = x_h @ ctx2_h ; out_b = blood_h @ ctx1_h

Refactor (Gram trick):
    k_h^T v_h = W_k_h (tok^T tok) W_v_h^T  with G = tok^T tok  [C, C]
so the N=4096 contraction happens once (G), and everything downstream is tiny
[C,C]-scale work.  ctxT blocks come from Q = G @ WkT then per head-pair
WvT_pair^T @ Q_pair whose diagonal 64x64 blocks are ctxT_h.  Softmax along the
free axis; normalized probs land in the diagonal blocks of a zeroed [128,128]
tile F; BD = F^T (PE transpose) is the block-diagonal ctx pair used by
    out[n, (h,e)] = sum_{(h,d)} tokT[(h,d), n] * BD[(h,d), (h,e)]

All on-chip data is fp16 (PSUM accumulation stays f32): fp16 matmuls and
transposes run at 1 cycle/column on the PE, and fp16 HBM I/O halves DMA
traffic vs f32.  Host casts inputs to fp16 and upcasts the fp16 outputs;
SCALE (2^-3, exact) folds into the k-weights.

Schedule (v3): ONE joint accumulation phase computes G_x and G_b together
(6 PSUM banks: per stream m0, m1, m2+m3 packed) plus the x transposes on PE;
blood transposes go through the DMA crossbar (dma_start_transpose) straight
from DRAM into per-group SBUF tiles, off the PE's critical path.  Chain A
runs, then out_b production is interleaved with chain B's matmuls so the ob
writes and chain B share the window; finally out_x streams out.  Both 4 MB
output streams thus pipeline against live compute instead of draining at the
end.

Sharding: data-parallel over batch B=8 across the 8 cores; weights replicated.
"""

import sys

if "/opt/trn_rl_repo" not in sys.path:
    sys.path.insert(0, "/opt/trn_rl_repo")

import numpy as np

from concourse import bacc, masks, mybir, tile
from concourse.bass_utils import run_bass_kernel_spmd

B, N, C, H = 8, 4096, 512, 8
D = C // H
SCALE = D ** -0.5
P = 128
NBIG = N // 512          # 8 big row tiles (512 rows each)
NT = N // P              # 32 n-tiles
CB = C // P              # 4 column blocks == head pairs
F32 = mybir.dt.float32
F16 = mybir.dt.float16
AX = mybir.AxisListType
ACT_EXP = mybir.ActivationFunctionType.Exp

# G is symmetric: row-block m only needs columns >= G_OFF[m]
G_OFF = [0, P, 2 * P, 3 * P]

def build_nc():
    nc = bacc.Bacc("TRN2", target_bir_lowering=False, debug=False)

    xb = nc.dram_tensor("xb", [N, C], F16, kind="ExternalInput").ap()
    bb = nc.dram_tensor("bb", [N, C], F16, kind="ExternalInput").ap()
    w1t = nc.dram_tensor("w1t", [C, 2 * C], F16, kind="ExternalInput").ap()
    w2t = nc.dram_tensor("w2t", [C, 2 * C], F16, kind="ExternalInput").ap()
    # blocked transposed output layout: [kb, part(c within pair), pair, n-col]
    ox = nc.dram_tensor("oxT", [NBIG, P, CB, 512], F16, kind="ExternalOutput").ap()
    ob = nc.dram_tensor("obT", [NBIG, P, CB, 512], F16, kind="ExternalOutput").ap()

    with tile.TileContext(nc) as tc:
        _emit(nc, tc, xb, bb, w1t, w2t, ox, ob)

    nc.compile()
    return nc


def _emit(nc, tc, xb, bb, w1t, w2t, ox, ob):
    from contextlib import ExitStack

    ctx = ExitStack()
    with ctx:
        const = ctx.enter_context(tc.tile_pool(name="const", bufs=1))
        wpool = ctx.enter_context(tc.tile_pool(name="wpool", bufs=1))
        tokp = ctx.enter_context(tc.tile_pool(name="tokp", bufs=16))
        xtp = ctx.enter_context(tc.tile_pool(name="xtp", bufs=NBIG))
        xtbp = ctx.enter_context(tc.tile_pool(name="xtbp", bufs=NBIG))
        gqp = ctx.enter_context(tc.tile_pool(name="gqp", bufs=8))
        smallp = ctx.enter_context(tc.tile_pool(name="smallp", bufs=4))
        fpool = ctx.enter_context(tc.tile_pool(name="fpool", bufs=8))
        bdpool = ctx.enter_context(tc.tile_pool(name="bdpool", bufs=8))
        ostp = ctx.enter_context(tc.tile_pool(name="ostp", bufs=4))
        # PSUM: 3 banks G accumulators (x's slots are drained before blood's
        # allocations claim them), 3 banks out-psum rotation, 2 banks for
        # transposes / chain work.  3+3+2 = all 8 banks.
        psGa = ctx.enter_context(tc.tile_pool(name="psGa", bufs=3, space="PSUM"))
        psOut = ctx.enter_context(tc.tile_pool(name="psOut", bufs=3, space="PSUM"))
        psO = ctx.enter_context(tc.tile_pool(name="psO", bufs=2, space="PSUM"))

        ident = const.tile([P, P], F16, tag="idh")
        masks.make_identity(nc, ident[:])

        # weights: chunk j (c-rows 128j..128j+128) lives at cols [j*2C, (j+1)*2C)
        w_x = wpool.tile([P, CB * 2 * C], F16, tag="wx")
        w_b = wpool.tile([P, CB * 2 * C], F16, tag="wb")

        def load_weights():
            nc.sync.dma_start(
                w_x[:].rearrange("p (j c) -> p j c", j=CB),
                w1t[:, :].rearrange("(j p) c -> p j c", p=P),
            )
            nc.sync.dma_start(
                w_b[:].rearrange("p (j c) -> p j c", j=CB),
                w2t[:, :].rearrange("(j p) c -> p j c", p=P),
            )

        def wchunk(w, j):
            return w[:, j * 2 * C:(j + 1) * 2 * C]

        # transposed tokens: per-kb contiguous tiles (the xbar DMA transpose
        # needs a contiguous destination), pair block m at cols [m*512, ...)
        xT_x = [xtp.tile([P, CB * 512], F16, tag="xtx", name=f"xtx{kb}")
                for kb in range(NBIG)]
        xT_b = [xtbp.tile([P, CB * 512], F16, tag="xtb", name=f"xtb{kb}")
                for kb in range(NBIG)]

        def load_tok(toks, tok_dram, kb, split=False):
            tokb = tokp.tile([P, 4 * C], F16, tag="tok", name=f"tok{kb}")
            if split:
                for sub in range(4):
                    nc.sync.dma_start(
                        tokb[:, sub * C:(sub + 1) * C],
                        tok_dram[kb * 512 + sub * P:kb * 512 + (sub + 1) * P, :],
                    )
            else:
                nc.sync.dma_start(
                    tokb[:].rearrange("p (s c) -> p s c", s=4),
                    tok_dram[kb * 512:(kb + 1) * 512, :].rearrange(
                        "(s p) c -> p s c", p=P
                    ),
                )
            toks.append(tokb)

        def emit_G_tile(gps, sb, k):
            # gps = [m0 tile (cols 0:512), m1 tile (cols 0:384 = G cols
            # 128:512), m2 tile (cols 0:256 = G cols 256:512)]; the m3
            # [128,128] block gets its own accumulation pass later (a PSUM
            # bank supports only one open accumulation group at a time).
            st, sp = (k == 0), (k == NT - 1)
            nc.tensor.matmul(gps[0][:, 0:C], sb[:, 0:P], sb[:, 0:C],
                             start=st, stop=sp)
            nc.tensor.matmul(gps[1][:, 0:C - P], sb[:, P:2 * P], sb[:, P:C],
                             start=st, stop=sp)
            nc.tensor.matmul(gps[2][:, 0:2 * P], sb[:, 2 * P:3 * P],
                             sb[:, 2 * P:C], start=st, stop=sp)

        def emit_G3(toks):
            """deferred G(3,3) block: one [128,128] accumulation over all
            32 n-tiles, in a psO slot (runs after W1 frees the ring)."""
            g3 = psO.tile([P, P], F32, tag="o", name="g3")
            for k in range(NT):
                sb = toks[k // 4][:, (k % 4) * C + 3 * P:(k % 4) * C + C]
                nc.tensor.matmul(g3[:], sb[:], sb[:],
                                 start=(k == 0), stop=(k == NT - 1))
            return g3

        def emit_T_tile(dst3, sb, k, ncol):
            """transpose [128n, 512c] sub-tile k into dst3 [p, m, ncol] view"""
            tps = psO.tile([P, C], F16, tag="o", name="tps")
            for m in range(CB):
                nc.tensor.transpose(
                    tps[:, m * P:(m + 1) * P], sb[:, m * P:(m + 1) * P], ident[:],
                )
            kk = k % (ncol // P)
            dst = dst3[:, :, kk * P:(kk + 1) * P]
            src = tps[:].rearrange("p (m n) -> p m n", m=CB)
            if k % 2:
                nc.scalar.copy(dst, src)
            else:
                nc.vector.tensor_copy(dst, src)

        def out_chunk(rhs_fn, BDs, kb, odram, ptag):
            """outT for 512 n-cols (tile-group kb): per pair p one matmul
            [c-block p, 512 n]; drain (split DVE/ACT), one batched write per
            kb with the issuing engine alternating so a trigger waiting on
            the other engine's drain never blocks a queue for long."""
            ost = ostp.tile([P, 4 * 512], F16, tag="ost", name="ost")
            for p in range(CB):
                ops = psOut.tile([P, 512], F32, tag=ptag, name=f"ops{p}")
                nc.tensor.matmul(ops[:], BDs[p][:], rhs_fn(p),
                                 start=True, stop=True)
                if p % 2:
                    nc.scalar.copy(ost[:, p * 512:(p + 1) * 512], ops[:])
                else:
                    nc.vector.tensor_copy(ost[:, p * 512:(p + 1) * 512], ops[:])
            # two writes per group (pairs 01, pairs 23), issuing engine
            # alternating per half: each write starts as soon as its two
            # drains land, and no single engine queue serializes the stream
            o3 = ost[:].rearrange("p (c n) -> p c n", c=CB)
            nc.sync.dma_start(odram[kb, :, 0:2, :], o3[:, 0:2, :])
            nc.scalar.dma_start(odram[kb, :, 2:4, :], o3[:, 2:4, :])

        def chain_drain_mirror(gps, g3, vec_drain=False):
            """G psum -> g_sb fp16 with mirrored lower blocks."""
            g_sb = [gqp.tile([P, C], F16, tag="gq", name=f"g{m}")
                    for m in range(CB)]
            cp = nc.vector.tensor_copy if vec_drain else nc.scalar.copy
            cp(g_sb[0][:, 0:C], gps[0][:, 0:C])
            cp(g_sb[1][:, P:C], gps[1][:, 0:C - P])
            cp(g_sb[2][:, 2 * P:C], gps[2][:, 0:2 * P])
            cp(g_sb[3][:, 3 * P:C], g3[:])
            nmir = 0
            for i in range(CB):
                for j in range(G_OFF[i] // P):
                    mps = psO.tile([P, P], F16, tag="o", name="mps")
                    nc.tensor.transpose(
                        mps[:], g_sb[j][:, i * P:(i + 1) * P], ident[:],
                    )
                    if nmir % 2:
                        nc.scalar.copy(g_sb[i][:, j * P:(j + 1) * P], mps[:])
                    else:
                        nc.vector.tensor_copy(
                            g_sb[i][:, j * P:(j + 1) * P], mps[:])
                    nmir += 1
            return g_sb

        def chain_Q(g_sb, w, i):
            qp = psO.tile([P, C], F32, tag="o", name=f"qp{i}")
            for j in range(CB):
                nc.tensor.matmul(
                    qp[:], g_sb[j][:, i * P:(i + 1) * P],
                    wchunk(w, j)[:, 0:C], start=(j == 0), stop=(j == 3),
                )
            q = gqp.tile([P, C], F16, tag="gq", name=f"q{i}")
            nc.scalar.copy(q[:], qp[:])
            return q

        def chain_ctx(q_sb, w, p):
            """ctx psum for pair p -> softmax -> normalized F tile (fp16)."""
            cps = psO.tile([P, P], F32, tag="o", name=f"cps{p}")
            for j in range(CB):
                nc.tensor.matmul(
                    cps[:],
                    wchunk(w, j)[:, C + p * P:C + (p + 1) * P],
                    q_sb[j][:, p * P:(p + 1) * P],
                    start=(j == 0), stop=(j == 3),
                )
            nm = smallp.tile([P, 1], F32, tag="nm", name="nm")
            sm = smallp.tile([P, 1], F32, tag="sm", name="sm")
            rv = smallp.tile([P, 1], F32, tag="rv", name="rv")
            pp = smallp.tile([P, D], F32, tag="pp", name="pp")
            fp = fpool.tile([P, P], F16, tag="F", name="fp")
            nc.gpsimd.memset(fp[:], 0.0)
            for dd in range(2):
                s0 = slice(dd * D, (dd + 1) * D)
                blk = cps[s0, s0]
                nc.vector.reduce_max(nm[s0, :], blk, axis=AX.X, negate=True)
                nc.scalar.activation(
                    pp[s0, :], blk, ACT_EXP, bias=nm[s0, :], scale=1.0,
                    accum_out=sm[s0, :],
                )
            nc.vector.reciprocal(rv[:], sm[:])
            for dd in range(2):
                s0 = slice(dd * D, (dd + 1) * D)
                nc.vector.tensor_scalar_mul(fp[s0, s0], pp[s0, :], rv[s0, :])
            return fp

        def chain_bd(Fs):
            BDs = []
            for p in range(CB):
                bps = psO.tile([P, P], F16, tag="o", name="bps")
                nc.tensor.transpose(bps[:], Fs[p][:], ident[:])
                bd = bdpool.tile([P, P], F16, tag="bd", name=f"bd{p}")
                nc.vector.tensor_copy(bd[:], bps[:])
                BDs.append(bd)
            return BDs

        # ---- schedule ----
        # loads first (they own the DMA queues early), then the x-stream
        # crossbar transposes (queued behind the loads)
        toks_x, toks_b = [], []
        load_tok(toks_x, xb, 0, split=True)
        for kb in range(1, NBIG):
            load_tok(toks_x, xb, kb)
        load_weights()
        for kb in range(NBIG):
            load_tok(toks_b, bb, kb)

        # phase A: G_x accumulation + x transposes
        gx = [psGa.tile([P, C], F32, tag="g", name=f"gx{m}") for m in range(3)]
        for kb in range(NBIG):
            for sub in range(4):
                k = kb * 4 + sub
                sx = toks_x[kb][:, sub * C:(sub + 1) * C]
                emit_G_tile(gx, sx, k)
                emit_T_tile(
                    xT_x[kb][:].rearrange("p (m n) -> p m n", m=CB),
                    sx, k, 512)
        g3x = emit_G3(toks_x)

        # chain A (x's ctx -> BD1); fill the softmax bubble with the first
        # blood tile-group's transposes + G_b start
        g1 = chain_drain_mirror(gx, g3x)
        q1 = [chain_Q(g1, w_x, i) for i in range(CB)]
        Fs1 = [chain_ctx(q1, w_x, p) for p in range(CB)]
        gb = [psGa.tile([P, C], F32, tag="g", name=f"gb{m}") for m in range(3)]
        for sub in range(4):
            sb_ = toks_b[0][:, sub * C:(sub + 1) * C]
            emit_T_tile(
                xT_b[0][:].rearrange("p (m n) -> p m n", m=CB), sb_, sub, 512)
            emit_G_tile(gb, sb_, sub)
        bd1 = chain_bd(Fs1)

        # phase B: per 512-row group: transpose blood, accumulate G_b, and
        # produce out_b for the previous group (writes stream out early)
        for kb in range(1, NBIG):
            for sub in range(4):
                k = kb * 4 + sub
                sb_ = toks_b[kb][:, sub * C:(sub + 1) * C]
                emit_T_tile(
                    xT_b[kb][:].rearrange("p (m n) -> p m n", m=CB),
                    sb_, k, 512)
                emit_G_tile(gb, sb_, k)
            if kb < NBIG - 1:
                out_chunk(
                    lambda p, kb=kb - 1: xT_b[kb][:, p * 512:(p + 1) * 512],
                    bd1, kb - 1, ob, "g")
        g3b = emit_G3(toks_b)

        # chain B; the last two out_b groups fill its softmax window
        g2 = chain_drain_mirror(gb, g3b, True)
        q2 = [chain_Q(g2, w_b, i) for i in range(CB)]
        out_chunk(lambda p: xT_b[NBIG - 2][:, p * 512:(p + 1) * 512],
                  bd1, NBIG - 2, ob, "g")
        Fs2 = [chain_ctx(q2, w_b, p) for p in range(2)]
        out_chunk(lambda p: xT_b[NBIG - 1][:, p * 512:(p + 1) * 512],
                  bd1, NBIG - 1, ob, "g")
        Fs2 += [chain_ctx(q2, w_b, p) for p in range(2, CB)]
        bd2 = chain_bd(Fs2)

        # phase C: out_x production (~0.5 MB per group, pipelined writes)
        for kb in range(NBIG):
            out_chunk(
                lambda p, kb=kb: xT_x[kb][:, p * 512:(p + 1) * 512],
                bd2, kb, ox, "g")


_NC_CACHE = None


def _get_nc():
    global _NC_CACHE
    if _NC_CACHE is None:
        _NC_CACHE = build_nc()
    return _NC_CACHE


def _prep_inputs(x, blood, W1, W2):
    x16 = np.ascontiguousarray(np.asarray(x, dtype=np.float32).astype(np.float16))
    b16 = np.ascontiguousarray(
        np.asarray(blood, dtype=np.float32).astype(np.float16))
    w1t = np.ascontiguousarray(np.asarray(W1, dtype=np.float32).T)
    w2t = np.ascontiguousarray(np.asarray(W2, dtype=np.float32).T)
    w1t[:, :C] *= SCALE  # fold softmax scale into the k-projection (exact: 2^-3)
    w2t[:, :C] *= SCALE
    w1t = w1t.astype(np.float16)
    w2t = w2t.astype(np.float16)
    return [
        {"xb": x16[b], "bb": b16[b], "w1t": w1t, "w2t": w2t} for b in range(B)
    ]


def _unshuffle(arr):
    """[NBIG, P, CB, 512] blocked-transposed fp16 -> [N, C] f32 natural."""
    # arr[kb, part, p, col] = out[kb*512 + col, p*128 + part]
    return np.ascontiguousarray(
        arr.transpose(0, 3, 2, 1).reshape(N, C).astype(np.float32))


def kernel(x, blood, W1, W2, trace=False):
    nc = _get_nc()
    in_maps = _prep_inputs(x, blood, W1, W2)
    res = run_bass_kernel_spmd(nc, in_maps, core_ids=list(range(B)), trace=trace)
    out_x = np.stack([_unshuffle(res.results[b]["oxT"]) for b in range(B)])
    out_b = np.stack([_unshuffle(res.results[b]["obT"]) for b in range(B)])
    if trace:
        kernel.last_results = res
    return (out_x, out_b)
